# revision 22
# baseline (speedup 1.0000x reference)
"""AtomPlacementScheduler Trainium2 kernel.

out[b] = sum_e irfft(rfft(stems[b,e]) * exp(-2i pi f s_be)),  s = sigmoid(TL@W+b)*N.

4-step FFT (N = 32768 = 256 x 128) with all heavy work on the TensorEngine.

v2 over the session-1 baseline:
 - Stage-1 exploits real-input Hermitian symmetry of the inner DFT: only
   k2 = 0..128 columns are computed (258 vs 512 free cols), and the upper
   half k2 = 129..255 is reconstructed at the m12 step via column-reversed
   access patterns against p1sb = [re(129) | im(129) | -im(127, for the
   reversed reads)].  Tensor stage-1 cost drops ~40%.
 - The C = twiddle*shift-phase block ships as TWO plain blocks [Cre | Cim]
   (the old record shipped four sign-arranged copies).  The m12 product
   tile [p1re*Cre | p1re*Cim | p1im*Cre | p1im*Cim] is built by reading
   p1sb and the C block through stride-0 broadcast dims, and the one
   negative sign (Ure = p1re*Cre - p1im*Cim) is applied by a negated
   stationary [-Mre|-Mim] in a third stage-3 matmul.  Record shrinks
   360KB -> 256KB per event.
 - Stage-3 accumulates X across the 16 events in PSUM as before; the
   inverse per batch is unchanged except st2 groups matmuls by stationary
   (4 loads instead of 8).
 - PE warm-up runs on memzero'd SBUF tiles so it needs no DMA and starts
   immediately, covering the initial const+record DMA latency.

Pure data parallel over batch: 64 batches / 8 cores = 8 per core.
Self-contained: hardcodes shapes B=64, E=16, N=32768, n_cores=8.
"""
import numpy as np
import ml_dtypes

N = 32768
N1 = 128   # outer DFT size (n1, k1)
N2 = 256   # inner DFT size (n2, k2)
E = 16
B = 64
NCORES = 8
BC = B // NCORES      # 8 batches per core
S = BC * E            # 128 signals per core
K1 = 64               # k1 = 0..63; DC/Nyquist corrections are exact on host
K2H = 129             # Hermitian inner-DFT cols: k2 = 0..128
# record: 256 stems | 256 Cre | 256 Cim | 128 [Mre|Mim] | 128 [-Mre|-Mim]
RECW = 1024

F32 = np.float32
BF16 = ml_dtypes.bfloat16


def _host_consts():
    n1 = np.arange(N1)
    n2 = np.arange(N2)
    k2 = np.arange(N2)
    k1 = np.arange(K1)
    W2 = np.exp(-2j * np.pi * np.outer(n2, k2) / N2)            # (n2, k2)
    W2cat = np.concatenate([W2.real[:, :K2H], W2.imag[:, :K2H]], 1)  # (256, 258)
    E1 = np.exp(+2j * np.pi * np.outer(k1[:64], n1) / N1)       # (k1<64, m)
    # partition-stacked inverse stage-2 movings: the stage-3 PSUM tile absb
    # holds [Mre^T U (rows 0:64); Mim^T U (rows 64:128)], so a matmul with a
    # full-128-partition stationary absb-chunk against these stacked movings
    # performs the Xre/Xim combines for free:
    #   Gre = statR @ F1 + statI @ F2,  Gim = statR @ F3 + statI @ F1
    e1cat = np.zeros((128, 384))
    e1cat[0:64, 0:128] = E1.real      # F1 = [E1re; -E1im]
    e1cat[64:128, 0:128] = -E1.imag
    e1cat[0:64, 128:256] = -E1.imag   # F2 = [-E1im; -E1re]
    e1cat[64:128, 128:256] = -E1.real
    e1cat[0:64, 256:384] = E1.imag    # F3 = [E1im; E1re]
    e1cat[64:128, 256:384] = E1.real
    TinvT = np.exp(+2j * np.pi * np.outer(k2, n1) / N)          # (k2, m)
    tinv = np.zeros((2, 128, 256))
    for c in range(2):
        tinv[c, :, 0:128] = TinvT.real[c * 128:(c + 1) * 128]
        tinv[c, :, 128:256] = TinvT.imag[c * 128:(c + 1) * 128]
    E2 = np.exp(+2j * np.pi * np.outer(k2, n2) / N2) * (2.0 / N)  # (k2, n2)
    e2 = np.zeros((2, 128, 512))
    for c in range(2):
        e2[c, :, 0:256] = E2.real[c * 128:(c + 1) * 128]
        e2[c, :, 256:512] = -E2.imag[c * 128:(c + 1) * 128]
    return W2cat, e1cat, tinv, e2


def _build_graph():
    import concourse.bass as bass
    import concourse.mybir as mybir
    import concourse.tile as tile
    from concourse import bacc
    from concourse.ap import AP

    dt = mybir.dt
    nc = bacc.Bacc("TRN2", target_bir_lowering=False, debug=False, num_devices=NCORES)

    rec_d = nc.dram_tensor("rec", [BC, E, 128, RECW], dt.bfloat16, kind="ExternalInput")
    w2_d = nc.dram_tensor("w2cat", [N2, 2 * K2H], dt.bfloat16, kind="ExternalInput")
    e1_d = nc.dram_tensor("e1cat", [128, 384], dt.bfloat16, kind="ExternalInput")
    tinv_d = nc.dram_tensor("tinv", [2, 128, 256], dt.bfloat16, kind="ExternalInput")
    e2_d = nc.dram_tensor("e2", [2, 128, 512], dt.bfloat16, kind="ExternalInput")
    out_d = nc.dram_tensor("out", [BC, N2, N1], dt.float32, kind="ExternalOutput")

    LAG = 3

    with tile.TileContext(nc) as tc:
        with (
            tc.tile_pool(name="const", bufs=1) as cpool,
            tc.tile_pool(name="rec", bufs=LAG + 9) as recpool,
            tc.tile_pool(name="work", bufs=10) as pool,
            tc.tile_pool(name="inv", bufs=2) as ipool,
            tc.tile_pool(name="p1p", bufs=3, space="PSUM") as p1pool,
            tc.tile_pool(name="pxp", bufs=2, space="PSUM") as pxpool,
            tc.tile_pool(name="pgp", bufs=2, space="PSUM") as pgpool,
            tc.tile_pool(name="pyp", bufs=1, space="PSUM") as pypool,
        ):
            w2_0 = cpool.tile([128, 2 * K2H], dt.bfloat16, tag="w2_0")
            w2_1 = cpool.tile([128, 2 * K2H], dt.bfloat16, tag="w2_1")
            nc.sync.dma_start(w2_0[:], w2_d[0:128, :])
            nc.sync.dma_start(w2_1[:], w2_d[128:256, :])
            e1 = cpool.tile([128, 384], dt.bfloat16, tag="e1")
            nc.sync.dma_start(e1[:], e1_d[:])
            tinv_0 = cpool.tile([128, 256], dt.bfloat16, tag="tinv0")
            tinv_1 = cpool.tile([128, 256], dt.bfloat16, tag="tinv1")
            nc.sync.dma_start(tinv_0[:], tinv_d[0])
            nc.sync.dma_start(tinv_1[:], tinv_d[1])
            e2_0 = cpool.tile([128, 512], dt.bfloat16, tag="e2_0")
            e2_1 = cpool.tile([128, 512], dt.bfloat16, tag="e2_1")
            nc.sync.dma_start(e2_0[:], e2_d[0])
            nc.sync.dma_start(e2_1[:], e2_d[1])
            tinv = [tinv_0, tinv_1]
            e2t = [e2_0, e2_1]
            w2 = [w2_0, w2_1]

            # HAM warm-up: ~5us of back-to-back dummy matmuls un-throttle the
            # PE clock gate (4/8 -> 8/8, 1.2 -> 2.4 GHz).  Runs on a
            # gpsimd-memset SBUF tile so it starts immediately (no DMA
            # dependency) and covers the const + first-record DMA latency.
            wz_m = cpool.tile([128, 512], dt.bfloat16, tag="wz_m")
            nc.gpsimd.memset(wz_m[:], 0.0)
            negone = cpool.tile([128, 8], dt.bfloat16, tag="negone")
            nc.gpsimd.memset(negone[:], -1.0)
            pwarm = pypool.tile([128, 512], dt.float32, tag="pY", name="pwarm")
            for _ in range(16):
                nc.tensor.matmul(pwarm[:], wz_m[:, 0:128], wz_m[:], start=True, stop=True)

            slots = {}

            def front(i):
                b, e = divmod(i, E)
                rec = recpool.tile([128, RECW], dt.bfloat16, tag="rec")
                nc.sync.dma_start(rec[:], rec_d[b, e])
                p1 = p1pool.tile([128, 2 * K2H], dt.float32, tag="p1")
                nc.tensor.matmul(p1[:], rec[:, 0:128], w2[0][:], start=True, stop=False)
                nc.tensor.matmul(p1[:], rec[:, 128:256], w2[1][:], start=False, stop=True)
                slots[i] = (rec, p1)

            def back(i, step):
                b, e = divmod(i, E)
                rec, p1 = slots.pop(i)
                # p1sb = [re(129) | im(129) | -im[1..127] (127)]
                p1sb = pool.tile([128, 385], dt.bfloat16, tag="p1sb")
                nc.scalar.copy(p1sb[:, 0:258], p1[:])
                nc.gpsimd.tensor_mul(
                    p1sb[:, 258:385], p1sb[:, 130:257],
                    negone[:, 0:1].broadcast_to([128, 127]))
                # m12 = [p1re*Cre | p1re*Cim | p1im*Cre | p1im*Cim], with the
                # full 256-col k2 range rebuilt from the Hermitian half:
                # p1{re,im}[k2>=129] come from column-reversed reads (the im
                # part from the pre-negated third region of p1sb).
                m12 = pool.tile([128, 1024], dt.bfloat16, tag="m12")
                m12v = m12[:].rearrange("p (b r c) -> p b r c", b=2, r=2)
                p1F = (p1sb[:, 0:258].rearrange("p (b c) -> p b c", b=2)
                       .unsqueeze(2).broadcast_to([128, 2, 2, K2H]))
                bse = p1sb[:]
                p1R = AP(bse.tensor, bse.offset + 127,
                         [list(bse.ap[0]), [257, 2], [0, 2], [-1, 127]])
                cq = (rec[:, 256:768].rearrange("p (r c) -> p r c", r=2)
                      .unsqueeze(1).broadcast_to([128, 2, 2, 256]))
                nc.vector.tensor_mul(m12v[:, :, :, 0:K2H], p1F, cq[:, :, :, 0:K2H])
                nc.vector.tensor_mul(m12v[:, :, :, K2H:256], p1R, cq[:, :, :, K2H:256])
                if e == 0:
                    slots[("pX", b)] = pxpool.tile([128, 512], dt.float32,
                                                   tag="pAB", name="pAB")
                pAB = slots[("pX", b)]
                # Stage 3: pAB[:, 0:256] = M^T(p1re*Cre) - M^T(p1im*Cim),
                #          pAB[:, 256:512] = M^T(p1re*Cim) + M^T(p1im*Cre);
                # the minus rides the shipped negated stationary Mng.
                Mst = rec[:, 768:896]
                Mng = rec[:, 896:1024]
                if e == 0:
                    nc.tensor.matmul(pAB[:], Mst, m12[:, 0:512],
                                     start=True, stop=False)
                    nc.tensor.matmul(pAB[:, 256:512], Mst, m12[:, 512:768],
                                     start=False, stop=False)
                    nc.tensor.matmul(pAB[:, 0:256], Mng, m12[:, 768:1024],
                                     start=False, stop=False)
                else:
                    nc.tensor.matmul(pAB[:, 0:256], Mng, m12[:, 768:1024],
                                     start=False, stop=False)
                    nc.tensor.matmul(pAB[:], Mst, m12[:, 0:512],
                                     start=False, stop=False)
                    nc.tensor.matmul(pAB[:, 256:512], Mst, m12[:, 512:768],
                                     start=False, stop=(e == E - 1))
                if e == E - 1:
                    inverse(i, b, slots.pop(("pX", b)), step)

            pending = {}

            def sched(step, fn):
                pending.setdefault(step, []).append(fn)

            def inverse(i, b, pX, step):
                pAB = pX
                absb = ipool.tile([128, 512], dt.bfloat16, tag="absb")
                pG = pgpool.tile([128, 512], dt.float32, tag="pG", name="pG")
                pY = pypool.tile([128, 512], dt.float32, tag="pY", name="pY")

                def st1():
                    nc.scalar.copy(absb[:], pAB[:])

                def st2():
                    # full-partition stationaries straight from absb; the
                    # stacked e1 movings perform the Xre/Xim combines inside
                    # the contraction (no partition-crossing DMA needed).
                    for c in range(2):
                        statR = absb[:, c * 128:(c + 1) * 128]
                        statI = absb[:, 256 + c * 128:256 + (c + 1) * 128]
                        o = c * 256
                        nc.tensor.matmul(pG[:, o:o + 128], statR, e1[:, 0:128],
                                         start=(c == 0), stop=False)
                        nc.tensor.matmul(pG[:, o + 128:o + 256], statR,
                                         e1[:, 256:384], start=False, stop=False)
                        nc.tensor.matmul(pG[:, o:o + 128], statI, e1[:, 128:256],
                                         start=False, stop=False)
                        nc.tensor.matmul(pG[:, o + 128:o + 256], statI,
                                         e1[:, 0:128], start=False, stop=(c == 1))

                gts = []

                def st3():
                    for c in range(2):
                        gsb = ipool.tile([128, 256], dt.bfloat16, tag=f"gsb{c}")
                        nc.scalar.copy(gsb[:], pG[:, c * 256:(c + 1) * 256])
                        g1 = ipool.tile([128, 128], dt.bfloat16, tag=f"g1{c}")
                        g2 = ipool.tile([128, 128], dt.bfloat16, tag=f"g2{c}")
                        g3 = ipool.tile([128, 128], dt.bfloat16, tag=f"g3{c}")
                        g4 = ipool.tile([128, 128], dt.bfloat16, tag=f"g4{c}")
                        gt = ipool.tile([128, 256], dt.bfloat16, tag=f"gt{c}")
                        nc.vector.tensor_mul(g1[:], gsb[:, 0:128],
                                             tinv[c][:, 0:128])
                        nc.vector.tensor_mul(g2[:], gsb[:, 128:256],
                                             tinv[c][:, 128:256])
                        nc.vector.tensor_sub(gt[:, 0:128], g1[:], g2[:])
                        nc.gpsimd.tensor_mul(g3[:], gsb[:, 0:128],
                                             tinv[c][:, 128:256])
                        nc.gpsimd.tensor_mul(g4[:], gsb[:, 128:256],
                                             tinv[c][:, 0:128])
                        nc.vector.tensor_add(gt[:, 128:256], g3[:], g4[:])
                        gts.append(gt)

                def st4():
                    for j in range(2):
                        nc.tensor.matmul(pY[:, j * 128:(j + 1) * 128],
                                         e2t[0][:, j * 128:(j + 1) * 128],
                                         gts[0][:, 0:128], start=(j == 0),
                                         stop=False)
                        nc.tensor.matmul(pY[:, j * 128:(j + 1) * 128],
                                         e2t[0][:, 256 + j * 128:256 + (j + 1) * 128],
                                         gts[0][:, 128:256], start=False,
                                         stop=False)
                        nc.tensor.matmul(pY[:, j * 128:(j + 1) * 128],
                                         e2t[1][:, j * 128:(j + 1) * 128],
                                         gts[1][:, 0:128], start=False,
                                         stop=False)
                        nc.tensor.matmul(pY[:, j * 128:(j + 1) * 128],
                                         e2t[1][:, 256 + j * 128:256 + (j + 1) * 128],
                                         gts[1][:, 128:256], start=False,
                                         stop=(j == 1))

                def st5():
                    for j in range(2):
                        ysb = ipool.tile([128, 128], dt.float32, tag=f"ysb{j}")
                        nc.scalar.copy(ysb[:], pY[:, j * 128:(j + 1) * 128])
                        nc.sync.dma_start(out_d[b, j * 128:(j + 1) * 128, :],
                                          ysb[:])

                # Spread the five inverse stages across subsequent pipeline
                # steps so their tensor-engine work interleaves with the next
                # events' stage-1/3 matmuls instead of head-of-line blocking
                # the tensor queue behind the scalar/vector inverse chain.
                sched(step + 1, st1)
                sched(step + 2, st2)
                sched(step + 3, st3)
                sched(step + 4, st4)
                sched(step + 5, st5)

            # Interleave batch pairs (b0e0, b1e0, b0e1, b1e1, ...) so each
            # batch's inverse overlaps the other's event stream instead of
            # stalling the pipeline at batch boundaries.
            def order(step):
                pair, w = divmod(step, 2 * E)
                e, o = divmod(w, 2)
                return (2 * pair + o) * E + e

            for t in range(S + LAG + 10):
                if t < S:
                    front(order(t))
                j = t - LAG
                if 0 <= j < S:
                    back(order(j), j)
                for fn in pending.pop(j, ()):
                    fn()

    nc.compile()
    return nc


def kernel(time_latent, stems, targets, W_pos, b_pos):
    from concourse.bass_utils import run_bass_kernel_spmd

    # host: positions (tiny linear+sigmoid, fp32 exactly like the reference)
    z = np.einsum("bed,od->beo", time_latent.astype(F32), W_pos.astype(F32))
    z = z.reshape(B, E) + b_pos.reshape(1)[0]
    pos = 1.0 / (1.0 + np.exp(-z, dtype=F32))
    s = (pos * np.float32(N)).astype(np.float64)

    W2cat, e1cat, tinv, e2 = _host_consts()
    n1 = np.arange(N1)
    k2 = np.arange(N2)
    k1 = np.arange(K1)
    T = np.exp(-2j * np.pi * np.outer(n1, k2) / N)   # (n1, k2)
    W1 = np.exp(-2j * np.pi * np.outer(n1, k1) / N1)  # (n1, k1)

    w2cat_b = W2cat.astype(BF16)
    e1cat_b = e1cat.astype(BF16)
    tinv_b = tinv.astype(BF16)
    e2_b = e2.astype(BF16)

    nc = _build_graph()
    in_maps = []
    for c in range(NCORES):
        sl = slice(c * BC, (c + 1) * BC)
        s_flat = s[sl].reshape(-1)                                   # (S,)
        rec = np.empty((S, 128, RECW), dtype=BF16)
        # stems: (S, 256, 128) -> (S, 2, 128, 128) -> (S, 128, 2, 128)
        st = stems[sl].reshape(S, 2, 128, 128).transpose(0, 2, 1, 3)
        rec[:, :, 0:256] = st.reshape(S, 128, 256).astype(BF16)
        A = np.exp(-2j * np.pi * np.outer(s_flat, k2) / N)           # (S, k2)
        C = T[None, :, :] * A[:, None, :]                            # (S, n1, k2)
        rec[:, :, 256:512] = C.real.astype(BF16)
        rec[:, :, 512:768] = C.imag.astype(BF16)
        del C
        Bt = np.exp(-2j * np.pi * np.outer(s_flat, k1) / N1)         # (S, k1)
        M = W1[None, :, :] * Bt[:, None, :]                          # (S, n1, k1)
        Mcat = np.concatenate([M.real, M.imag], 2).astype(BF16)      # (S, 128, 128)
        rec[:, :, 768:896] = Mcat
        rec[:, :, 896:1024] = -Mcat
        del M, Mcat
        in_maps.append({
            "rec": rec.reshape(BC, E, 128, RECW),
            "w2cat": w2cat_b,
            "e1cat": e1cat_b,
            "tinv": tinv_b,
            "e2": e2_b,
        })

    import os
    trace = bool(int(os.environ.get("ATHENA_TRACE", "0")))
    res = run_bass_kernel_spmd(nc, in_maps, core_ids=list(range(NCORES)), trace=trace)
    if trace:
        print(f"HW exec time: {res.exec_time_ns} ns")

    # exact DC / Nyquist corrections on host (the device spectrum covers
    # k1 = 0..63 only): X0 = sum_e sum_n x, XNyq = sum_e cos(pi s) sum_n x(-1)^n
    sign = np.where(np.arange(N) % 2 == 0, 1.0, -1.0).astype(F32)
    sums = stems.astype(F32).sum(axis=2)                       # (B, E)
    dots = (stems.astype(F32) * sign[None, None, :]).sum(axis=2)
    X0 = sums.sum(axis=1)                                      # (B,)
    XNyq = (np.cos(np.pi * s) * dots).sum(axis=1).astype(F32)  # (B,)

    outs = []
    for c in range(NCORES):
        sl = slice(c * BC, (c + 1) * BC)
        y = res.results[c]["out"].reshape(BC, N).astype(F32)
        y = y + (-X0[sl, None].astype(F32)
                 + sign[None, :] * XNyq[sl, None]) / np.float32(N)
        outs.append(y)
    return np.concatenate(outs, 0).reshape(B, 1, N).astype(F32)


# revision 23
# speedup vs baseline: 1.0936x; 1.0936x over previous
"""AtomPlacementScheduler Trainium2 kernel.

out[b] = sum_e irfft(rfft(stems[b,e]) * exp(-2i pi f s_be)),  s = sigmoid(TL@W+b)*N.

4-step FFT (N = 32768 = 256 x 128) with all heavy work on the TensorEngine.

v2 over the session-1 baseline:
 - Stage-1 exploits real-input Hermitian symmetry of the inner DFT: only
   k2 = 0..128 columns are computed (258 vs 512 free cols), and the upper
   half k2 = 129..255 is reconstructed at the m12 step via column-reversed
   access patterns against p1sb = [re(129) | im(129) | -im(127, for the
   reversed reads)].  Tensor stage-1 cost drops ~40%.
 - The C = twiddle*shift-phase block ships as TWO plain blocks [Cre | Cim]
   (the old record shipped four sign-arranged copies).  The m12 product
   tile [p1re*Cre | p1re*Cim | p1im*Cre | p1im*Cim] is built by reading
   p1sb and the C block through stride-0 broadcast dims, and the one
   negative sign (Ure = p1re*Cre - p1im*Cim) is applied by a negated
   stationary [-Mre|-Mim] in a third stage-3 matmul.  Record shrinks
   360KB -> 256KB per event.
 - Stage-3 accumulates X across the 16 events in PSUM as before; the
   inverse per batch is unchanged except st2 groups matmuls by stationary
   (4 loads instead of 8).
 - PE warm-up runs on memzero'd SBUF tiles so it needs no DMA and starts
   immediately, covering the initial const+record DMA latency.

Pure data parallel over batch: 64 batches / 8 cores = 8 per core.
Self-contained: hardcodes shapes B=64, E=16, N=32768, n_cores=8.
"""
import numpy as np
import ml_dtypes

N = 32768
N1 = 128   # outer DFT size (n1, k1)
N2 = 256   # inner DFT size (n2, k2)
E = 16
B = 64
NCORES = 8
BC = B // NCORES      # 8 batches per core
S = BC * E            # 128 signals per core
K1 = 64               # k1 = 0..63; DC/Nyquist corrections are exact on host
K2H = 129             # Hermitian inner-DFT cols: k2 = 0..128
# record: 256 stems | 256 Cre | 256 Cim | 128 [Mre|Mim] | 128 [-Mre|-Mim]
RECW = 1024

F32 = np.float32
BF16 = ml_dtypes.bfloat16


def _host_consts():
    n1 = np.arange(N1)
    n2 = np.arange(N2)
    k2 = np.arange(N2)
    k1 = np.arange(K1)
    W2 = np.exp(-2j * np.pi * np.outer(n2, k2) / N2)            # (n2, k2)
    W2cat = np.concatenate([W2.real[:, :K2H], W2.imag[:, :K2H]], 1)  # (256, 258)
    E1 = np.exp(+2j * np.pi * np.outer(k1[:64], n1) / N1)       # (k1<64, m)
    # partition-stacked inverse stage-2 movings: the stage-3 PSUM tile absb
    # holds [Mre^T U (rows 0:64); Mim^T U (rows 64:128)], so a matmul with a
    # full-128-partition stationary absb-chunk against these stacked movings
    # performs the Xre/Xim combines for free:
    #   Gre = statR @ F1 + statI @ F2,  Gim = statR @ F3 + statI @ F1
    e1cat = np.zeros((128, 384))
    e1cat[0:64, 0:128] = E1.real      # F1 = [E1re; -E1im]
    e1cat[64:128, 0:128] = -E1.imag
    e1cat[0:64, 128:256] = -E1.imag   # F2 = [-E1im; -E1re]
    e1cat[64:128, 128:256] = -E1.real
    e1cat[0:64, 256:384] = E1.imag    # F3 = [E1im; E1re]
    e1cat[64:128, 256:384] = E1.real
    TinvT = np.exp(+2j * np.pi * np.outer(k2, n1) / N)          # (k2, m)
    tinv = np.zeros((2, 128, 256))
    for c in range(2):
        tinv[c, :, 0:128] = TinvT.real[c * 128:(c + 1) * 128]
        tinv[c, :, 128:256] = TinvT.imag[c * 128:(c + 1) * 128]
    E2 = np.exp(+2j * np.pi * np.outer(k2, n2) / N2) * (2.0 / N)  # (k2, n2)
    e2 = np.zeros((2, 128, 512))
    for c in range(2):
        e2[c, :, 0:256] = E2.real[c * 128:(c + 1) * 128]
        e2[c, :, 256:512] = -E2.imag[c * 128:(c + 1) * 128]
    return W2cat, e1cat, tinv, e2


def _build_graph():
    import concourse.bass as bass
    import concourse.mybir as mybir
    import concourse.tile as tile
    from concourse import bacc
    from concourse.ap import AP

    dt = mybir.dt
    nc = bacc.Bacc("TRN2", target_bir_lowering=False, debug=False, num_devices=NCORES)

    rec_d = nc.dram_tensor("rec", [BC, E, 128, RECW], dt.bfloat16, kind="ExternalInput")
    w2_d = nc.dram_tensor("w2cat", [N2, 2 * K2H], dt.bfloat16, kind="ExternalInput")
    e1_d = nc.dram_tensor("e1cat", [128, 384], dt.bfloat16, kind="ExternalInput")
    tinv_d = nc.dram_tensor("tinv", [2, 128, 256], dt.bfloat16, kind="ExternalInput")
    e2_d = nc.dram_tensor("e2", [2, 128, 512], dt.bfloat16, kind="ExternalInput")
    out_d = nc.dram_tensor("out", [BC, N2, N1], dt.float32, kind="ExternalOutput")

    LAG = 3

    with tile.TileContext(nc) as tc:
        with (
            tc.tile_pool(name="const", bufs=1) as cpool,
            tc.tile_pool(name="rec", bufs=LAG + 9) as recpool,
            tc.tile_pool(name="work", bufs=10) as pool,
            tc.tile_pool(name="inv", bufs=2) as ipool,
            tc.tile_pool(name="p1p", bufs=3, space="PSUM") as p1pool,
            tc.tile_pool(name="pxp", bufs=2, space="PSUM") as pxpool,
            tc.tile_pool(name="pgp", bufs=2, space="PSUM") as pgpool,
            tc.tile_pool(name="pyp", bufs=1, space="PSUM") as pypool,
        ):
            w2_0 = cpool.tile([128, 2 * K2H], dt.bfloat16, tag="w2_0")
            w2_1 = cpool.tile([128, 2 * K2H], dt.bfloat16, tag="w2_1")
            nc.sync.dma_start(w2_0[:], w2_d[0:128, :])
            nc.sync.dma_start(w2_1[:], w2_d[128:256, :])
            e1 = cpool.tile([128, 384], dt.bfloat16, tag="e1")
            nc.sync.dma_start(e1[:], e1_d[:])
            tinv_0 = cpool.tile([128, 256], dt.bfloat16, tag="tinv0")
            tinv_1 = cpool.tile([128, 256], dt.bfloat16, tag="tinv1")
            nc.sync.dma_start(tinv_0[:], tinv_d[0])
            nc.sync.dma_start(tinv_1[:], tinv_d[1])
            e2_0 = cpool.tile([128, 512], dt.bfloat16, tag="e2_0")
            e2_1 = cpool.tile([128, 512], dt.bfloat16, tag="e2_1")
            nc.sync.dma_start(e2_0[:], e2_d[0])
            nc.sync.dma_start(e2_1[:], e2_d[1])
            tinv = [tinv_0, tinv_1]
            e2t = [e2_0, e2_1]
            w2 = [w2_0, w2_1]

            # HAM warm-up: ~5us of back-to-back dummy matmuls un-throttle the
            # PE clock gate (4/8 -> 8/8, 1.2 -> 2.4 GHz).  Runs on a
            # gpsimd-memset SBUF tile so it starts immediately (no DMA
            # dependency) and covers the const + first-record DMA latency.
            wz_m = cpool.tile([128, 512], dt.bfloat16, tag="wz_m")
            nc.gpsimd.memset(wz_m[:], 0.0)
            pwarm = pypool.tile([128, 512], dt.float32, tag="pY", name="pwarm")
            for _ in range(16):
                nc.tensor.matmul(pwarm[:], wz_m[:, 0:128], wz_m[:], start=True, stop=True)

            slots = {}

            def front(i):
                b, e = divmod(i, E)
                rec = recpool.tile([128, RECW], dt.bfloat16, tag="rec")
                nc.sync.dma_start(rec[:], rec_d[b, e])
                p1 = p1pool.tile([128, 2 * K2H], dt.float32, tag="p1")
                nc.tensor.matmul(p1[:], rec[:, 0:128], w2[0][:], start=True, stop=False)
                nc.tensor.matmul(p1[:], rec[:, 128:256], w2[1][:], start=False, stop=True)
                slots[i] = (rec, p1)

            def back(i, step):
                b, e = divmod(i, E)
                rec, p1 = slots.pop(i)
                # p1sb = [re(129) | im(129) | -im[1..127] (127)]
                p1sb = pool.tile([128, 385], dt.bfloat16, tag="p1sb")
                nc.scalar.copy(p1sb[:, 0:258], p1[:])
                nc.scalar.mul(p1sb[:, 258:385], p1[:, 130:257], -1.0)
                # m12 = [p1re*Cre | p1re*Cim | p1im*Cre | p1im*Cim], with the
                # full 256-col k2 range rebuilt from the Hermitian half:
                # p1{re,im}[k2>=129] come from column-reversed reads (the im
                # part from the pre-negated third region of p1sb).
                m12 = pool.tile([128, 1024], dt.bfloat16, tag="m12")
                m12v = m12[:].rearrange("p (b r c) -> p b r c", b=2, r=2)
                p1F = (p1sb[:, 0:258].rearrange("p (b c) -> p b c", b=2)
                       .unsqueeze(2).broadcast_to([128, 2, 2, K2H]))
                bse = p1sb[:]
                p1R = AP(bse.tensor, bse.offset + 127,
                         [list(bse.ap[0]), [257, 2], [0, 2], [-1, 127]])
                cq = (rec[:, 256:768].rearrange("p (r c) -> p r c", r=2)
                      .unsqueeze(1).broadcast_to([128, 2, 2, 256]))
                nc.vector.tensor_mul(m12v[:, :, :, 0:K2H], p1F, cq[:, :, :, 0:K2H])
                nc.vector.tensor_mul(m12v[:, :, :, K2H:256], p1R, cq[:, :, :, K2H:256])
                if e == 0:
                    slots[("pX", b)] = pxpool.tile([128, 512], dt.float32,
                                                   tag="pAB", name="pAB")
                pAB = slots[("pX", b)]
                # Stage 3: pAB[:, 0:256] = M^T(p1re*Cre) - M^T(p1im*Cim),
                #          pAB[:, 256:512] = M^T(p1re*Cim) + M^T(p1im*Cre);
                # the minus rides the shipped negated stationary Mng.
                Mst = rec[:, 768:896]
                Mng = rec[:, 896:1024]
                if e == 0:
                    nc.tensor.matmul(pAB[:], Mst, m12[:, 0:512],
                                     start=True, stop=False)
                    nc.tensor.matmul(pAB[:, 256:512], Mst, m12[:, 512:768],
                                     start=False, stop=False)
                    nc.tensor.matmul(pAB[:, 0:256], Mng, m12[:, 768:1024],
                                     start=False, stop=False)
                else:
                    nc.tensor.matmul(pAB[:, 0:256], Mng, m12[:, 768:1024],
                                     start=False, stop=False)
                    nc.tensor.matmul(pAB[:], Mst, m12[:, 0:512],
                                     start=False, stop=False)
                    nc.tensor.matmul(pAB[:, 256:512], Mst, m12[:, 512:768],
                                     start=False, stop=(e == E - 1))
                if e == E - 1:
                    inverse(i, b, slots.pop(("pX", b)), step)

            pending = {}

            def sched(step, fn):
                pending.setdefault(step, []).append(fn)

            def inverse(i, b, pX, step):
                pAB = pX
                absb = ipool.tile([128, 512], dt.bfloat16, tag="absb")
                pG = pgpool.tile([128, 512], dt.float32, tag="pG", name="pG")
                pY = pypool.tile([128, 512], dt.float32, tag="pY", name="pY")

                def st1():
                    nc.scalar.copy(absb[:], pAB[:])

                def st2():
                    # full-partition stationaries straight from absb; the
                    # stacked e1 movings perform the Xre/Xim combines inside
                    # the contraction (no partition-crossing DMA needed).
                    for c in range(2):
                        statR = absb[:, c * 128:(c + 1) * 128]
                        statI = absb[:, 256 + c * 128:256 + (c + 1) * 128]
                        o = c * 256
                        nc.tensor.matmul(pG[:, o:o + 128], statR, e1[:, 0:128],
                                         start=(c == 0), stop=False)
                        nc.tensor.matmul(pG[:, o + 128:o + 256], statR,
                                         e1[:, 256:384], start=False, stop=False)
                        nc.tensor.matmul(pG[:, o:o + 128], statI, e1[:, 128:256],
                                         start=False, stop=False)
                        nc.tensor.matmul(pG[:, o + 128:o + 256], statI,
                                         e1[:, 0:128], start=False, stop=(c == 1))

                gts = []

                def st3():
                    for c in range(2):
                        gsb = ipool.tile([128, 256], dt.bfloat16, tag=f"gsb{c}")
                        nc.scalar.copy(gsb[:], pG[:, c * 256:(c + 1) * 256])
                        g1 = ipool.tile([128, 128], dt.bfloat16, tag=f"g1{c}")
                        g2 = ipool.tile([128, 128], dt.bfloat16, tag=f"g2{c}")
                        g3 = ipool.tile([128, 128], dt.bfloat16, tag=f"g3{c}")
                        g4 = ipool.tile([128, 128], dt.bfloat16, tag=f"g4{c}")
                        gt = ipool.tile([128, 256], dt.bfloat16, tag=f"gt{c}")
                        nc.vector.tensor_mul(g1[:], gsb[:, 0:128],
                                             tinv[c][:, 0:128])
                        nc.vector.tensor_mul(g2[:], gsb[:, 128:256],
                                             tinv[c][:, 128:256])
                        nc.vector.tensor_sub(gt[:, 0:128], g1[:], g2[:])
                        nc.gpsimd.tensor_mul(g3[:], gsb[:, 0:128],
                                             tinv[c][:, 128:256])
                        nc.gpsimd.tensor_mul(g4[:], gsb[:, 128:256],
                                             tinv[c][:, 0:128])
                        nc.vector.tensor_add(gt[:, 128:256], g3[:], g4[:])
                        gts.append(gt)

                def st4():
                    for j in range(2):
                        nc.tensor.matmul(pY[:, j * 128:(j + 1) * 128],
                                         e2t[0][:, j * 128:(j + 1) * 128],
                                         gts[0][:, 0:128], start=(j == 0),
                                         stop=False)
                        nc.tensor.matmul(pY[:, j * 128:(j + 1) * 128],
                                         e2t[0][:, 256 + j * 128:256 + (j + 1) * 128],
                                         gts[0][:, 128:256], start=False,
                                         stop=False)
                        nc.tensor.matmul(pY[:, j * 128:(j + 1) * 128],
                                         e2t[1][:, j * 128:(j + 1) * 128],
                                         gts[1][:, 0:128], start=False,
                                         stop=False)
                        nc.tensor.matmul(pY[:, j * 128:(j + 1) * 128],
                                         e2t[1][:, 256 + j * 128:256 + (j + 1) * 128],
                                         gts[1][:, 128:256], start=False,
                                         stop=(j == 1))

                def st5():
                    for j in range(2):
                        ysb = ipool.tile([128, 128], dt.float32, tag=f"ysb{j}")
                        nc.scalar.copy(ysb[:], pY[:, j * 128:(j + 1) * 128])
                        nc.sync.dma_start(out_d[b, j * 128:(j + 1) * 128, :],
                                          ysb[:])

                # Spread the five inverse stages across subsequent pipeline
                # steps so their tensor-engine work interleaves with the next
                # events' stage-1/3 matmuls instead of head-of-line blocking
                # the tensor queue behind the scalar/vector inverse chain.
                sched(step + 1, st1)
                sched(step + 2, st2)
                sched(step + 3, st3)
                sched(step + 4, st4)
                sched(step + 5, st5)

            # Staggered schedule: batch b's events sit at slots
            # 16b + 2e + (b&1) (even/odd slot parity alternates per batch), so
            # consecutive batches overlap half their windows but completions
            # land 16 slots apart -- exactly one inverse in flight at a time,
            # its stages spread over the next batch's event stream.
            slot_ev = {}
            for b in range(BC):
                for e in range(E):
                    slot_ev[16 * b + 2 * e + (b & 1)] = b * E + e
            nslots = max(slot_ev) + 1
            for t in range(nslots + LAG + 10):
                if t in slot_ev:
                    front(slot_ev[t])
                j = t - LAG
                if j in slot_ev:
                    back(slot_ev[j], j)
                for fn in pending.pop(j, ()):
                    fn()

    nc.compile()
    return nc


def kernel(time_latent, stems, targets, W_pos, b_pos):
    from concourse.bass_utils import run_bass_kernel_spmd

    # host: positions (tiny linear+sigmoid, fp32 exactly like the reference)
    z = np.einsum("bed,od->beo", time_latent.astype(F32), W_pos.astype(F32))
    z = z.reshape(B, E) + b_pos.reshape(1)[0]
    pos = 1.0 / (1.0 + np.exp(-z, dtype=F32))
    s = (pos * np.float32(N)).astype(np.float64)

    W2cat, e1cat, tinv, e2 = _host_consts()
    n1 = np.arange(N1)
    k2 = np.arange(N2)
    k1 = np.arange(K1)
    T = np.exp(-2j * np.pi * np.outer(n1, k2) / N)   # (n1, k2)
    W1 = np.exp(-2j * np.pi * np.outer(n1, k1) / N1)  # (n1, k1)

    w2cat_b = W2cat.astype(BF16)
    e1cat_b = e1cat.astype(BF16)
    tinv_b = tinv.astype(BF16)
    e2_b = e2.astype(BF16)

    nc = _build_graph()
    in_maps = []
    for c in range(NCORES):
        sl = slice(c * BC, (c + 1) * BC)
        s_flat = s[sl].reshape(-1)                                   # (S,)
        rec = np.empty((S, 128, RECW), dtype=BF16)
        # stems: (S, 256, 128) -> (S, 2, 128, 128) -> (S, 128, 2, 128)
        st = stems[sl].reshape(S, 2, 128, 128).transpose(0, 2, 1, 3)
        rec[:, :, 0:256] = st.reshape(S, 128, 256).astype(BF16)
        A = np.exp(-2j * np.pi * np.outer(s_flat, k2) / N)           # (S, k2)
        C = T[None, :, :] * A[:, None, :]                            # (S, n1, k2)
        rec[:, :, 256:512] = C.real.astype(BF16)
        rec[:, :, 512:768] = C.imag.astype(BF16)
        del C
        Bt = np.exp(-2j * np.pi * np.outer(s_flat, k1) / N1)         # (S, k1)
        M = W1[None, :, :] * Bt[:, None, :]                          # (S, n1, k1)
        Mcat = np.concatenate([M.real, M.imag], 2).astype(BF16)      # (S, 128, 128)
        rec[:, :, 768:896] = Mcat
        rec[:, :, 896:1024] = -Mcat
        del M, Mcat
        in_maps.append({
            "rec": rec.reshape(BC, E, 128, RECW),
            "w2cat": w2cat_b,
            "e1cat": e1cat_b,
            "tinv": tinv_b,
            "e2": e2_b,
        })

    import os
    trace = bool(int(os.environ.get("ATHENA_TRACE", "0")))
    res = run_bass_kernel_spmd(nc, in_maps, core_ids=list(range(NCORES)), trace=trace)
    if trace:
        print(f"HW exec time: {res.exec_time_ns} ns")

    # exact DC / Nyquist corrections on host (the device spectrum covers
    # k1 = 0..63 only): X0 = sum_e sum_n x, XNyq = sum_e cos(pi s) sum_n x(-1)^n
    sign = np.where(np.arange(N) % 2 == 0, 1.0, -1.0).astype(F32)
    sums = stems.astype(F32).sum(axis=2)                       # (B, E)
    dots = (stems.astype(F32) * sign[None, None, :]).sum(axis=2)
    X0 = sums.sum(axis=1)                                      # (B,)
    XNyq = (np.cos(np.pi * s) * dots).sum(axis=1).astype(F32)  # (B,)

    outs = []
    for c in range(NCORES):
        sl = slice(c * BC, (c + 1) * BC)
        y = res.results[c]["out"].reshape(BC, N).astype(F32)
        y = y + (-X0[sl, None].astype(F32)
                 + sign[None, :] * XNyq[sl, None]) / np.float32(N)
        outs.append(y)
    return np.concatenate(outs, 0).reshape(B, 1, N).astype(F32)


# revision 24
# speedup vs baseline: 1.1318x; 1.0349x over previous
"""AtomPlacementScheduler Trainium2 kernel.

out[b] = sum_e irfft(rfft(stems[b,e]) * exp(-2i pi f s_be)),  s = sigmoid(TL@W+b)*N.

4-step FFT (N = 32768 = 256 x 128) with all heavy work on the TensorEngine.

v2 over the session-1 baseline:
 - Stage-1 exploits real-input Hermitian symmetry of the inner DFT: only
   k2 = 0..128 columns are computed (258 vs 512 free cols), and the upper
   half k2 = 129..255 is reconstructed at the m12 step via column-reversed
   access patterns against p1sb = [re(129) | im(129) | -im(127, for the
   reversed reads)].  Tensor stage-1 cost drops ~40%.
 - The C = twiddle*shift-phase block ships as TWO plain blocks [Cre | Cim]
   (the old record shipped four sign-arranged copies).  The m12 product
   tile [p1re*Cre | p1re*Cim | p1im*Cre | p1im*Cim] is built by reading
   p1sb and the C block through stride-0 broadcast dims, and the one
   negative sign (Ure = p1re*Cre - p1im*Cim) is applied by a negated
   stationary [-Mre|-Mim] in a third stage-3 matmul.  Record shrinks
   360KB -> 256KB per event.
 - Stage-3 accumulates X across the 16 events in PSUM as before; the
   inverse per batch is unchanged except st2 groups matmuls by stationary
   (4 loads instead of 8).
 - PE warm-up runs on memzero'd SBUF tiles so it needs no DMA and starts
   immediately, covering the initial const+record DMA latency.

Pure data parallel over batch: 64 batches / 8 cores = 8 per core.
Self-contained: hardcodes shapes B=64, E=16, N=32768, n_cores=8.
"""
import numpy as np
import ml_dtypes

N = 32768
N1 = 128   # outer DFT size (n1, k1)
N2 = 256   # inner DFT size (n2, k2)
E = 16
B = 64
NCORES = 8
BC = B // NCORES      # 8 batches per core
S = BC * E            # 128 signals per core
K1 = 64               # k1 = 0..63; DC/Nyquist corrections are exact on host
K2H = 129             # Hermitian inner-DFT cols: k2 = 0..128
# record: 256 stems | 256 Cre | 256 Cim | 128 [Mre|Mim] | 128 [-Mre|-Mim]
RECW = 1024

F32 = np.float32
BF16 = ml_dtypes.bfloat16


def _host_consts():
    n1 = np.arange(N1)
    n2 = np.arange(N2)
    k2 = np.arange(N2)
    k1 = np.arange(K1)
    W2 = np.exp(-2j * np.pi * np.outer(n2, k2) / N2)            # (n2, k2)
    W2cat = np.concatenate([W2.real[:, :K2H], W2.imag[:, :K2H]], 1)  # (256, 258)
    E1 = np.exp(+2j * np.pi * np.outer(k1[:64], n1) / N1)       # (k1<64, m)
    # partition-stacked inverse stage-2 movings: the stage-3 PSUM tile absb
    # holds [Mre^T U (rows 0:64); Mim^T U (rows 64:128)], so a matmul with a
    # full-128-partition stationary absb-chunk against these stacked movings
    # performs the Xre/Xim combines for free:
    #   Gre = statR @ F1 + statI @ F2,  Gim = statR @ F3 + statI @ F1
    e1cat = np.zeros((128, 384))
    e1cat[0:64, 0:128] = E1.real      # F1 = [E1re; -E1im]
    e1cat[64:128, 0:128] = -E1.imag
    e1cat[0:64, 128:256] = -E1.imag   # F2 = [-E1im; -E1re]
    e1cat[64:128, 128:256] = -E1.real
    e1cat[0:64, 256:384] = E1.imag    # F3 = [E1im; E1re]
    e1cat[64:128, 256:384] = E1.real
    TinvT = np.exp(+2j * np.pi * np.outer(k2, n1) / N)          # (k2, m)
    tinv = np.zeros((2, 128, 256))
    for c in range(2):
        tinv[c, :, 0:128] = TinvT.real[c * 128:(c + 1) * 128]
        tinv[c, :, 128:256] = TinvT.imag[c * 128:(c + 1) * 128]
    E2 = np.exp(+2j * np.pi * np.outer(k2, n2) / N2) * (2.0 / N)  # (k2, n2)
    e2 = np.zeros((2, 128, 512))
    for c in range(2):
        e2[c, :, 0:256] = E2.real[c * 128:(c + 1) * 128]
        e2[c, :, 256:512] = -E2.imag[c * 128:(c + 1) * 128]
    return W2cat, e1cat, tinv, e2


def _build_graph():
    import concourse.bass as bass
    import concourse.mybir as mybir
    import concourse.tile as tile
    from concourse import bacc
    from concourse.ap import AP

    dt = mybir.dt
    nc = bacc.Bacc("TRN2", target_bir_lowering=False, debug=False, num_devices=NCORES)

    rec_d = nc.dram_tensor("rec", [BC, E, 128, RECW], dt.bfloat16, kind="ExternalInput")
    w2_d = nc.dram_tensor("w2cat", [N2, 2 * K2H], dt.bfloat16, kind="ExternalInput")
    e1_d = nc.dram_tensor("e1cat", [128, 384], dt.bfloat16, kind="ExternalInput")
    tinv_d = nc.dram_tensor("tinv", [2, 128, 256], dt.bfloat16, kind="ExternalInput")
    e2_d = nc.dram_tensor("e2", [2, 128, 512], dt.bfloat16, kind="ExternalInput")
    out_d = nc.dram_tensor("out", [BC, N2, N1], dt.float32, kind="ExternalOutput")

    LAG = 3

    with tile.TileContext(nc) as tc:
        with (
            tc.tile_pool(name="const", bufs=1) as cpool,
            tc.tile_pool(name="rec", bufs=LAG + 9) as recpool,
            tc.tile_pool(name="work", bufs=10) as pool,
            tc.tile_pool(name="inv", bufs=2) as ipool,
            tc.tile_pool(name="p1p", bufs=3, space="PSUM") as p1pool,
            tc.tile_pool(name="pxp", bufs=2, space="PSUM") as pxpool,
            tc.tile_pool(name="pgp", bufs=2, space="PSUM") as pgpool,
            tc.tile_pool(name="pyp", bufs=1, space="PSUM") as pypool,
        ):
            w2_0 = cpool.tile([128, 2 * K2H], dt.bfloat16, tag="w2_0")
            w2_1 = cpool.tile([128, 2 * K2H], dt.bfloat16, tag="w2_1")
            nc.sync.dma_start(w2_0[:], w2_d[0:128, :])
            nc.sync.dma_start(w2_1[:], w2_d[128:256, :])
            e1 = cpool.tile([128, 384], dt.bfloat16, tag="e1")
            nc.sync.dma_start(e1[:], e1_d[:])
            tinv_0 = cpool.tile([128, 256], dt.bfloat16, tag="tinv0")
            tinv_1 = cpool.tile([128, 256], dt.bfloat16, tag="tinv1")
            nc.sync.dma_start(tinv_0[:], tinv_d[0])
            nc.sync.dma_start(tinv_1[:], tinv_d[1])
            e2_0 = cpool.tile([128, 512], dt.bfloat16, tag="e2_0")
            e2_1 = cpool.tile([128, 512], dt.bfloat16, tag="e2_1")
            nc.sync.dma_start(e2_0[:], e2_d[0])
            nc.sync.dma_start(e2_1[:], e2_d[1])
            tinv = [tinv_0, tinv_1]
            e2t = [e2_0, e2_1]
            w2 = [w2_0, w2_1]

            # HAM warm-up: ~5us of back-to-back dummy matmuls un-throttle the
            # PE clock gate (4/8 -> 8/8, 1.2 -> 2.4 GHz).  Runs on a
            # gpsimd-memset SBUF tile so it starts immediately (no DMA
            # dependency) and covers the const + first-record DMA latency.
            wz_m = cpool.tile([128, 512], dt.bfloat16, tag="wz_m")
            nc.gpsimd.memset(wz_m[:], 0.0)
            pwarm = pypool.tile([128, 512], dt.float32, tag="pY", name="pwarm")
            for _ in range(16):
                nc.tensor.matmul(pwarm[:], wz_m[:, 0:128], wz_m[:], start=True, stop=True)

            slots = {}

            def front(i):
                b, e = divmod(i, E)
                rec = recpool.tile([128, RECW], dt.bfloat16, tag="rec")
                nc.sync.dma_start(rec[:], rec_d[b, e])
                p1 = p1pool.tile([128, 2 * K2H], dt.float32, tag="p1")
                nc.tensor.matmul(p1[:], rec[:, 0:128], w2[0][:], start=True, stop=False)
                nc.tensor.matmul(p1[:], rec[:, 128:256], w2[1][:], start=False, stop=True)
                slots[i] = (rec, p1)

            def back(i, step):
                b, e = divmod(i, E)
                rec, p1 = slots.pop(i)
                # p1sb = [re(129) | im(129) | -im[1..127] (127)]
                p1sb = pool.tile([128, 385], dt.bfloat16, tag="p1sb")
                nc.scalar.copy(p1sb[:, 0:258], p1[:])
                nc.scalar.mul(p1sb[:, 258:385], p1[:, 130:257], -1.0)
                # m12 = [p1re*Cre | p1re*Cim | p1im*Cre | p1im*Cim], with the
                # full 256-col k2 range rebuilt from the Hermitian half:
                # p1{re,im}[k2>=129] come from column-reversed reads (the im
                # part from the pre-negated third region of p1sb).
                m12 = pool.tile([128, 1024], dt.bfloat16, tag="m12")
                m12v = m12[:].rearrange("p (b r c) -> p b r c", b=2, r=2)
                p1F = (p1sb[:, 0:258].rearrange("p (b c) -> p b c", b=2)
                       .unsqueeze(2).broadcast_to([128, 2, 2, K2H]))
                bse = p1sb[:]
                p1R = AP(bse.tensor, bse.offset + 127,
                         [list(bse.ap[0]), [257, 2], [0, 2], [-1, 127]])
                cq = (rec[:, 256:768].rearrange("p (r c) -> p r c", r=2)
                      .unsqueeze(1).broadcast_to([128, 2, 2, 256]))
                nc.vector.tensor_mul(m12v[:, :, :, 0:K2H], p1F, cq[:, :, :, 0:K2H])
                nc.vector.tensor_mul(m12v[:, :, :, K2H:256], p1R, cq[:, :, :, K2H:256])
                if e == 0:
                    slots[("pX", b)] = pxpool.tile([128, 512], dt.float32,
                                                   tag="pAB", name="pAB")
                pAB = slots[("pX", b)]
                # Stage 3: pAB[:, 0:256] = M^T(p1re*Cre) - M^T(p1im*Cim),
                #          pAB[:, 256:512] = M^T(p1re*Cim) + M^T(p1im*Cre);
                # the minus rides the shipped negated stationary Mng.
                Mst = rec[:, 768:896]
                Mng = rec[:, 896:1024]
                if e == 0:
                    nc.tensor.matmul(pAB[:], Mst, m12[:, 0:512],
                                     start=True, stop=False)
                    nc.tensor.matmul(pAB[:, 256:512], Mst, m12[:, 512:768],
                                     start=False, stop=False)
                    nc.tensor.matmul(pAB[:, 0:256], Mng, m12[:, 768:1024],
                                     start=False, stop=False)
                else:
                    nc.tensor.matmul(pAB[:, 0:256], Mng, m12[:, 768:1024],
                                     start=False, stop=False)
                    nc.tensor.matmul(pAB[:], Mst, m12[:, 0:512],
                                     start=False, stop=False)
                    nc.tensor.matmul(pAB[:, 256:512], Mst, m12[:, 512:768],
                                     start=False, stop=(e == E - 1))
                if e == E - 1:
                    inverse(i, b, slots.pop(("pX", b)), step)

            pending = {}

            def sched(step, fn):
                pending.setdefault(step, []).append(fn)

            def inverse(i, b, pX, step):
                pAB = pX
                absb = ipool.tile([128, 512], dt.bfloat16, tag="absb")
                pG = pgpool.tile([128, 512], dt.float32, tag="pG", name="pG")
                pY = pypool.tile([128, 512], dt.float32, tag="pY", name="pY")

                def st1():
                    nc.scalar.copy(absb[:], pAB[:])

                def st2():
                    # full-partition stationaries straight from absb; the
                    # stacked e1 movings perform the Xre/Xim combines inside
                    # the contraction (no partition-crossing DMA needed).
                    for c in range(2):
                        statR = absb[:, c * 128:(c + 1) * 128]
                        statI = absb[:, 256 + c * 128:256 + (c + 1) * 128]
                        o = c * 256
                        nc.tensor.matmul(pG[:, o:o + 128], statR, e1[:, 0:128],
                                         start=(c == 0), stop=False)
                        nc.tensor.matmul(pG[:, o + 128:o + 256], statR,
                                         e1[:, 256:384], start=False, stop=False)
                        nc.tensor.matmul(pG[:, o:o + 128], statI, e1[:, 128:256],
                                         start=False, stop=False)
                        nc.tensor.matmul(pG[:, o + 128:o + 256], statI,
                                         e1[:, 0:128], start=False, stop=(c == 1))

                gts = []

                def st3():
                    for c in range(2):
                        gsb = ipool.tile([128, 256], dt.bfloat16, tag=f"gsb{c}")
                        nc.scalar.copy(gsb[:], pG[:, c * 256:(c + 1) * 256])
                        g1 = ipool.tile([128, 128], dt.bfloat16, tag=f"g1{c}")
                        g2 = ipool.tile([128, 128], dt.bfloat16, tag=f"g2{c}")
                        g3 = ipool.tile([128, 128], dt.bfloat16, tag=f"g3{c}")
                        g4 = ipool.tile([128, 128], dt.bfloat16, tag=f"g4{c}")
                        gt = ipool.tile([128, 256], dt.bfloat16, tag=f"gt{c}")
                        nc.vector.tensor_mul(g1[:], gsb[:, 0:128],
                                             tinv[c][:, 0:128])
                        nc.vector.tensor_mul(g2[:], gsb[:, 128:256],
                                             tinv[c][:, 128:256])
                        nc.vector.tensor_sub(gt[:, 0:128], g1[:], g2[:])
                        nc.gpsimd.tensor_mul(g3[:], gsb[:, 0:128],
                                             tinv[c][:, 128:256])
                        nc.gpsimd.tensor_mul(g4[:], gsb[:, 128:256],
                                             tinv[c][:, 0:128])
                        nc.vector.tensor_add(gt[:, 128:256], g3[:], g4[:])
                        gts.append(gt)

                def st4():
                    for j in range(2):
                        nc.tensor.matmul(pY[:, j * 128:(j + 1) * 128],
                                         e2t[0][:, j * 128:(j + 1) * 128],
                                         gts[0][:, 0:128], start=(j == 0),
                                         stop=False)
                        nc.tensor.matmul(pY[:, j * 128:(j + 1) * 128],
                                         e2t[0][:, 256 + j * 128:256 + (j + 1) * 128],
                                         gts[0][:, 128:256], start=False,
                                         stop=False)
                        nc.tensor.matmul(pY[:, j * 128:(j + 1) * 128],
                                         e2t[1][:, j * 128:(j + 1) * 128],
                                         gts[1][:, 0:128], start=False,
                                         stop=False)
                        nc.tensor.matmul(pY[:, j * 128:(j + 1) * 128],
                                         e2t[1][:, 256 + j * 128:256 + (j + 1) * 128],
                                         gts[1][:, 128:256], start=False,
                                         stop=(j == 1))

                def st5():
                    for j in range(2):
                        ysb = ipool.tile([128, 128], dt.float32, tag=f"ysb{j}")
                        nc.scalar.copy(ysb[:], pY[:, j * 128:(j + 1) * 128])
                        nc.sync.dma_start(out_d[b, j * 128:(j + 1) * 128, :],
                                          ysb[:])

                # Spread the five inverse stages across subsequent pipeline
                # steps so their tensor-engine work interleaves with the next
                # events' stage-1/3 matmuls instead of head-of-line blocking
                # the tensor queue behind the scalar/vector inverse chain.
                sched(step + 1, st1)
                sched(step + 2, st2)
                sched(step + 3, st3)
                sched(step + 4, st4)
                sched(step + 5, st5)

            # Interleave batch pairs (b0e0, b1e0, b0e1, b1e1, ...) so each
            # batch's inverse overlaps the other's event stream instead of
            # stalling the pipeline at batch boundaries.
            def order(step):
                pair, w = divmod(step, 2 * E)
                e, o = divmod(w, 2)
                return (2 * pair + o) * E + e

            for t in range(S + LAG + 10):
                if t < S:
                    front(order(t))
                j = t - LAG
                if 0 <= j < S:
                    back(order(j), j)
                for fn in pending.pop(j, ()):
                    fn()

    nc.compile()
    return nc


def kernel(time_latent, stems, targets, W_pos, b_pos):
    from concourse.bass_utils import run_bass_kernel_spmd

    # host: positions (tiny linear+sigmoid, fp32 exactly like the reference)
    z = np.einsum("bed,od->beo", time_latent.astype(F32), W_pos.astype(F32))
    z = z.reshape(B, E) + b_pos.reshape(1)[0]
    pos = 1.0 / (1.0 + np.exp(-z, dtype=F32))
    s = (pos * np.float32(N)).astype(np.float64)

    W2cat, e1cat, tinv, e2 = _host_consts()
    n1 = np.arange(N1)
    k2 = np.arange(N2)
    k1 = np.arange(K1)
    T = np.exp(-2j * np.pi * np.outer(n1, k2) / N)   # (n1, k2)
    W1 = np.exp(-2j * np.pi * np.outer(n1, k1) / N1)  # (n1, k1)

    w2cat_b = W2cat.astype(BF16)
    e1cat_b = e1cat.astype(BF16)
    tinv_b = tinv.astype(BF16)
    e2_b = e2.astype(BF16)

    nc = _build_graph()
    in_maps = []
    for c in range(NCORES):
        sl = slice(c * BC, (c + 1) * BC)
        s_flat = s[sl].reshape(-1)                                   # (S,)
        rec = np.empty((S, 128, RECW), dtype=BF16)
        # stems: (S, 256, 128) -> (S, 2, 128, 128) -> (S, 128, 2, 128)
        st = stems[sl].reshape(S, 2, 128, 128).transpose(0, 2, 1, 3)
        rec[:, :, 0:256] = st.reshape(S, 128, 256).astype(BF16)
        A = np.exp(-2j * np.pi * np.outer(s_flat, k2) / N)           # (S, k2)
        C = T[None, :, :] * A[:, None, :]                            # (S, n1, k2)
        rec[:, :, 256:512] = C.real.astype(BF16)
        rec[:, :, 512:768] = C.imag.astype(BF16)
        del C
        Bt = np.exp(-2j * np.pi * np.outer(s_flat, k1) / N1)         # (S, k1)
        M = W1[None, :, :] * Bt[:, None, :]                          # (S, n1, k1)
        Mcat = np.concatenate([M.real, M.imag], 2).astype(BF16)      # (S, 128, 128)
        rec[:, :, 768:896] = Mcat
        rec[:, :, 896:1024] = -Mcat
        del M, Mcat
        in_maps.append({
            "rec": rec.reshape(BC, E, 128, RECW),
            "w2cat": w2cat_b,
            "e1cat": e1cat_b,
            "tinv": tinv_b,
            "e2": e2_b,
        })

    import os
    trace = bool(int(os.environ.get("ATHENA_TRACE", "0")))
    res = run_bass_kernel_spmd(nc, in_maps, core_ids=list(range(NCORES)), trace=trace)
    if trace:
        print(f"HW exec time: {res.exec_time_ns} ns")

    # exact DC / Nyquist corrections on host (the device spectrum covers
    # k1 = 0..63 only): X0 = sum_e sum_n x, XNyq = sum_e cos(pi s) sum_n x(-1)^n
    sign = np.where(np.arange(N) % 2 == 0, 1.0, -1.0).astype(F32)
    sums = stems.astype(F32).sum(axis=2)                       # (B, E)
    dots = (stems.astype(F32) * sign[None, None, :]).sum(axis=2)
    X0 = sums.sum(axis=1)                                      # (B,)
    XNyq = (np.cos(np.pi * s) * dots).sum(axis=1).astype(F32)  # (B,)

    outs = []
    for c in range(NCORES):
        sl = slice(c * BC, (c + 1) * BC)
        y = res.results[c]["out"].reshape(BC, N).astype(F32)
        y = y + (-X0[sl, None].astype(F32)
                 + sign[None, :] * XNyq[sl, None]) / np.float32(N)
        outs.append(y)
    return np.concatenate(outs, 0).reshape(B, 1, N).astype(F32)


# revision 25
# speedup vs baseline: 1.1531x; 1.0188x over previous
"""AtomPlacementScheduler Trainium2 kernel.

out[b] = sum_e irfft(rfft(stems[b,e]) * exp(-2i pi f s_be)),  s = sigmoid(TL@W+b)*N.

4-step FFT (N = 32768 = 256 x 128) with all heavy work on the TensorEngine.

v2 over the session-1 baseline:
 - Stage-1 exploits real-input Hermitian symmetry of the inner DFT: only
   k2 = 0..128 columns are computed (258 vs 512 free cols), and the upper
   half k2 = 129..255 is reconstructed at the m12 step via column-reversed
   access patterns against p1sb = [re(129) | im(129) | -im(127, for the
   reversed reads)].  Tensor stage-1 cost drops ~40%.
 - The C = twiddle*shift-phase block ships as TWO plain blocks [Cre | Cim]
   (the old record shipped four sign-arranged copies).  The m12 product
   tile [p1re*Cre | p1re*Cim | p1im*Cre | p1im*Cim] is built by reading
   p1sb and the C block through stride-0 broadcast dims, and the one
   negative sign (Ure = p1re*Cre - p1im*Cim) is applied by a negated
   stationary [-Mre|-Mim] in a third stage-3 matmul.  Record shrinks
   360KB -> 256KB per event.
 - Stage-3 accumulates X across the 16 events in PSUM as before; the
   inverse per batch is unchanged except st2 groups matmuls by stationary
   (4 loads instead of 8).
 - PE warm-up runs on memzero'd SBUF tiles so it needs no DMA and starts
   immediately, covering the initial const+record DMA latency.

Pure data parallel over batch: 64 batches / 8 cores = 8 per core.
Self-contained: hardcodes shapes B=64, E=16, N=32768, n_cores=8.
"""
import numpy as np
import ml_dtypes

N = 32768
N1 = 128   # outer DFT size (n1, k1)
N2 = 256   # inner DFT size (n2, k2)
E = 16
B = 64
NCORES = 8
BC = B // NCORES      # 8 batches per core
S = BC * E            # 128 signals per core
K1 = 64               # k1 = 0..63; DC/Nyquist corrections are exact on host
K2H = 129             # Hermitian inner-DFT cols: k2 = 0..128
# record: 256 stems | 256 Cre | 256 Cim | 128 [Mre|Mim] | 128 [-Mre|-Mim]
RECW = 1024

F32 = np.float32
BF16 = ml_dtypes.bfloat16


def _host_consts():
    n1 = np.arange(N1)
    n2 = np.arange(N2)
    k2 = np.arange(N2)
    k1 = np.arange(K1)
    W2 = np.exp(-2j * np.pi * np.outer(n2, k2) / N2)            # (n2, k2)
    W2cat = np.concatenate([W2.real[:, :K2H], W2.imag[:, :K2H]], 1)  # (256, 258)
    E1 = np.exp(+2j * np.pi * np.outer(k1[:64], n1) / N1)       # (k1<64, m)
    # partition-stacked inverse stage-2 movings: the stage-3 PSUM tile absb
    # holds [Mre^T U (rows 0:64); Mim^T U (rows 64:128)], so a matmul with a
    # full-128-partition stationary absb-chunk against these stacked movings
    # performs the Xre/Xim combines for free:
    #   Gre = statR @ F1 + statI @ F2,  Gim = statR @ F3 + statI @ F1
    e1cat = np.zeros((128, 384))
    e1cat[0:64, 0:128] = E1.real      # F1 = [E1re; -E1im]
    e1cat[64:128, 0:128] = -E1.imag
    e1cat[0:64, 128:256] = -E1.imag   # F2 = [-E1im; -E1re]
    e1cat[64:128, 128:256] = -E1.real
    e1cat[0:64, 256:384] = E1.imag    # F3 = [E1im; E1re]
    e1cat[64:128, 256:384] = E1.real
    TinvT = np.exp(+2j * np.pi * np.outer(k2, n1) / N)          # (k2, m)
    tinv = np.zeros((2, 128, 256))
    for c in range(2):
        tinv[c, :, 0:128] = TinvT.real[c * 128:(c + 1) * 128]
        tinv[c, :, 128:256] = TinvT.imag[c * 128:(c + 1) * 128]
    E2 = np.exp(+2j * np.pi * np.outer(k2, n2) / N2) * (2.0 / N)  # (k2, n2)
    e2 = np.zeros((2, 128, 512))
    for c in range(2):
        e2[c, :, 0:256] = E2.real[c * 128:(c + 1) * 128]
        e2[c, :, 256:512] = -E2.imag[c * 128:(c + 1) * 128]
    return W2cat, e1cat, tinv, e2


def _build_graph():
    import concourse.bass as bass
    import concourse.mybir as mybir
    import concourse.tile as tile
    from concourse import bacc
    from concourse.ap import AP

    dt = mybir.dt
    nc = bacc.Bacc("TRN2", target_bir_lowering=False, debug=False, num_devices=NCORES)

    rec_d = nc.dram_tensor("rec", [BC, E, 128, RECW], dt.bfloat16, kind="ExternalInput")
    w2_d = nc.dram_tensor("w2cat", [N2, 2 * K2H], dt.bfloat16, kind="ExternalInput")
    e1_d = nc.dram_tensor("e1cat", [128, 384], dt.bfloat16, kind="ExternalInput")
    tinv_d = nc.dram_tensor("tinv", [2, 128, 256], dt.bfloat16, kind="ExternalInput")
    e2_d = nc.dram_tensor("e2", [2, 128, 512], dt.bfloat16, kind="ExternalInput")
    out_d = nc.dram_tensor("out", [BC, N2, N1], dt.float32, kind="ExternalOutput")

    LAG = 3

    with tile.TileContext(nc) as tc:
        with (
            tc.tile_pool(name="const", bufs=1) as cpool,
            tc.tile_pool(name="rec", bufs=LAG + 9) as recpool,
            tc.tile_pool(name="work", bufs=10) as pool,
            tc.tile_pool(name="inv", bufs=2) as ipool,
            tc.tile_pool(name="p1p", bufs=3, space="PSUM") as p1pool,
            tc.tile_pool(name="pxp", bufs=2, space="PSUM") as pxpool,
            tc.tile_pool(name="pgp", bufs=2, space="PSUM") as pgpool,
            tc.tile_pool(name="pyp", bufs=1, space="PSUM") as pypool,
        ):
            w2_0 = cpool.tile([128, 2 * K2H], dt.bfloat16, tag="w2_0")
            w2_1 = cpool.tile([128, 2 * K2H], dt.bfloat16, tag="w2_1")
            nc.sync.dma_start(w2_0[:], w2_d[0:128, :])
            nc.sync.dma_start(w2_1[:], w2_d[128:256, :])
            e1 = cpool.tile([128, 384], dt.bfloat16, tag="e1")
            nc.sync.dma_start(e1[:], e1_d[:])
            tinv_0 = cpool.tile([128, 256], dt.bfloat16, tag="tinv0")
            tinv_1 = cpool.tile([128, 256], dt.bfloat16, tag="tinv1")
            nc.sync.dma_start(tinv_0[:], tinv_d[0])
            nc.sync.dma_start(tinv_1[:], tinv_d[1])
            e2_0 = cpool.tile([128, 512], dt.bfloat16, tag="e2_0")
            e2_1 = cpool.tile([128, 512], dt.bfloat16, tag="e2_1")
            nc.sync.dma_start(e2_0[:], e2_d[0])
            nc.sync.dma_start(e2_1[:], e2_d[1])
            tinv = [tinv_0, tinv_1]
            e2t = [e2_0, e2_1]
            w2 = [w2_0, w2_1]

            # HAM warm-up: ~5us of back-to-back dummy matmuls un-throttle the
            # PE clock gate (4/8 -> 8/8, 1.2 -> 2.4 GHz).  Runs on a
            # gpsimd-memset SBUF tile so it starts immediately (no DMA
            # dependency) and covers the const + first-record DMA latency.
            wz_m = cpool.tile([128, 512], dt.bfloat16, tag="wz_m")
            nc.gpsimd.memset(wz_m[:], 0.0)
            pwarm = pypool.tile([128, 512], dt.float32, tag="pY", name="pwarm")
            for _ in range(16):
                nc.tensor.matmul(pwarm[:], wz_m[:, 0:128], wz_m[:], start=True, stop=True)

            slots = {}

            def front(i):
                b, e = divmod(i, E)
                rec = recpool.tile([128, RECW], dt.bfloat16, tag="rec")
                nc.sync.dma_start(rec[:], rec_d[b, e])
                p1 = p1pool.tile([128, 2 * K2H], dt.float32, tag="p1")
                nc.tensor.matmul(p1[:], rec[:, 0:128], w2[0][:], start=True, stop=False)
                nc.tensor.matmul(p1[:], rec[:, 128:256], w2[1][:], start=False, stop=True)
                slots[i] = (rec, p1)

            def back(i, step):
                b, e = divmod(i, E)
                rec, p1 = slots.pop(i)
                # p1sb = [re(129) | im(129) | -im[1..127] (127)]
                p1sb = pool.tile([128, 385], dt.bfloat16, tag="p1sb")
                nc.scalar.copy(p1sb[:, 0:258], p1[:])
                nc.scalar.mul(p1sb[:, 258:385], p1sb[:, 130:257], -1.0)
                # m12 = [p1re*Cre | p1re*Cim | p1im*Cre | p1im*Cim], with the
                # full 256-col k2 range rebuilt from the Hermitian half:
                # p1{re,im}[k2>=129] come from column-reversed reads (the im
                # part from the pre-negated third region of p1sb).
                m12 = pool.tile([128, 1024], dt.bfloat16, tag="m12")
                m12v = m12[:].rearrange("p (b r c) -> p b r c", b=2, r=2)
                p1F = (p1sb[:, 0:258].rearrange("p (b c) -> p b c", b=2)
                       .unsqueeze(2).broadcast_to([128, 2, 2, K2H]))
                bse = p1sb[:]
                p1R = AP(bse.tensor, bse.offset + 127,
                         [list(bse.ap[0]), [257, 2], [0, 2], [-1, 127]])
                cq = (rec[:, 256:768].rearrange("p (r c) -> p r c", r=2)
                      .unsqueeze(1).broadcast_to([128, 2, 2, 256]))
                nc.vector.tensor_mul(m12v[:, :, :, 0:K2H], p1F, cq[:, :, :, 0:K2H])
                nc.vector.tensor_mul(m12v[:, :, :, K2H:256], p1R, cq[:, :, :, K2H:256])
                if e == 0:
                    slots[("pX", b)] = pxpool.tile([128, 512], dt.float32,
                                                   tag="pAB", name="pAB")
                pAB = slots[("pX", b)]
                # Stage 3: pAB[:, 0:256] = M^T(p1re*Cre) - M^T(p1im*Cim),
                #          pAB[:, 256:512] = M^T(p1re*Cim) + M^T(p1im*Cre);
                # the minus rides the shipped negated stationary Mng.
                Mst = rec[:, 768:896]
                Mng = rec[:, 896:1024]
                if e == 0:
                    nc.tensor.matmul(pAB[:], Mst, m12[:, 0:512],
                                     start=True, stop=False)
                    nc.tensor.matmul(pAB[:, 256:512], Mst, m12[:, 512:768],
                                     start=False, stop=False)
                    nc.tensor.matmul(pAB[:, 0:256], Mng, m12[:, 768:1024],
                                     start=False, stop=False)
                else:
                    nc.tensor.matmul(pAB[:, 0:256], Mng, m12[:, 768:1024],
                                     start=False, stop=False)
                    nc.tensor.matmul(pAB[:], Mst, m12[:, 0:512],
                                     start=False, stop=False)
                    nc.tensor.matmul(pAB[:, 256:512], Mst, m12[:, 512:768],
                                     start=False, stop=(e == E - 1))
                if e == E - 1:
                    inverse(i, b, slots.pop(("pX", b)), step)

            pending = {}

            def sched(step, fn):
                pending.setdefault(step, []).append(fn)

            def inverse(i, b, pX, step):
                pAB = pX
                absb = ipool.tile([128, 512], dt.bfloat16, tag="absb")
                pG = pgpool.tile([128, 512], dt.float32, tag="pG", name="pG")
                pY = pypool.tile([128, 512], dt.float32, tag="pY", name="pY")

                def st1():
                    nc.scalar.copy(absb[:], pAB[:])

                def st2():
                    # full-partition stationaries straight from absb; the
                    # stacked e1 movings perform the Xre/Xim combines inside
                    # the contraction (no partition-crossing DMA needed).
                    for c in range(2):
                        statR = absb[:, c * 128:(c + 1) * 128]
                        statI = absb[:, 256 + c * 128:256 + (c + 1) * 128]
                        o = c * 256
                        nc.tensor.matmul(pG[:, o:o + 128], statR, e1[:, 0:128],
                                         start=(c == 0), stop=False)
                        nc.tensor.matmul(pG[:, o + 128:o + 256], statR,
                                         e1[:, 256:384], start=False, stop=False)
                        nc.tensor.matmul(pG[:, o:o + 128], statI, e1[:, 128:256],
                                         start=False, stop=False)
                        nc.tensor.matmul(pG[:, o + 128:o + 256], statI,
                                         e1[:, 0:128], start=False, stop=(c == 1))

                gts = []

                def st3():
                    for c in range(2):
                        gsb = ipool.tile([128, 256], dt.bfloat16, tag=f"gsb{c}")
                        nc.scalar.copy(gsb[:], pG[:, c * 256:(c + 1) * 256])
                        g1 = ipool.tile([128, 128], dt.bfloat16, tag=f"g1{c}")
                        g2 = ipool.tile([128, 128], dt.bfloat16, tag=f"g2{c}")
                        g3 = ipool.tile([128, 128], dt.bfloat16, tag=f"g3{c}")
                        g4 = ipool.tile([128, 128], dt.bfloat16, tag=f"g4{c}")
                        gt = ipool.tile([128, 256], dt.bfloat16, tag=f"gt{c}")
                        nc.vector.tensor_mul(g1[:], gsb[:, 0:128],
                                             tinv[c][:, 0:128])
                        nc.vector.tensor_mul(g2[:], gsb[:, 128:256],
                                             tinv[c][:, 128:256])
                        nc.vector.tensor_sub(gt[:, 0:128], g1[:], g2[:])
                        nc.gpsimd.tensor_mul(g3[:], gsb[:, 0:128],
                                             tinv[c][:, 128:256])
                        nc.gpsimd.tensor_mul(g4[:], gsb[:, 128:256],
                                             tinv[c][:, 0:128])
                        nc.vector.tensor_add(gt[:, 128:256], g3[:], g4[:])
                        gts.append(gt)

                def st4():
                    for j in range(2):
                        nc.tensor.matmul(pY[:, j * 128:(j + 1) * 128],
                                         e2t[0][:, j * 128:(j + 1) * 128],
                                         gts[0][:, 0:128], start=(j == 0),
                                         stop=False)
                        nc.tensor.matmul(pY[:, j * 128:(j + 1) * 128],
                                         e2t[0][:, 256 + j * 128:256 + (j + 1) * 128],
                                         gts[0][:, 128:256], start=False,
                                         stop=False)
                        nc.tensor.matmul(pY[:, j * 128:(j + 1) * 128],
                                         e2t[1][:, j * 128:(j + 1) * 128],
                                         gts[1][:, 0:128], start=False,
                                         stop=False)
                        nc.tensor.matmul(pY[:, j * 128:(j + 1) * 128],
                                         e2t[1][:, 256 + j * 128:256 + (j + 1) * 128],
                                         gts[1][:, 128:256], start=False,
                                         stop=(j == 1))

                def st5():
                    for j in range(2):
                        ysb = ipool.tile([128, 128], dt.float32, tag=f"ysb{j}")
                        nc.scalar.copy(ysb[:], pY[:, j * 128:(j + 1) * 128])
                        nc.sync.dma_start(out_d[b, j * 128:(j + 1) * 128, :],
                                          ysb[:])

                # Spread the five inverse stages across subsequent pipeline
                # steps so their tensor-engine work interleaves with the next
                # events' stage-1/3 matmuls instead of head-of-line blocking
                # the tensor queue behind the scalar/vector inverse chain.
                if b >= BC - 2:
                    # tail: nothing left to overlap; run the chain densely
                    for fn in (st1, st2, st3, st4, st5):
                        sched(step + 1, fn)
                else:
                    sched(step + 1, st1)
                    sched(step + 2, st2)
                    sched(step + 3, st3)
                    sched(step + 4, st4)
                    sched(step + 5, st5)

            # Interleave batch pairs (b0e0, b1e0, b0e1, b1e1, ...) so each
            # batch's inverse overlaps the other's event stream instead of
            # stalling the pipeline at batch boundaries.
            def order(step):
                pair, w = divmod(step, 2 * E)
                e, o = divmod(w, 2)
                return (2 * pair + o) * E + e

            for t in range(S + LAG + 10):
                if t < S:
                    front(order(t))
                j = t - LAG
                if 0 <= j < S:
                    back(order(j), j)
                for fn in pending.pop(j, ()):
                    fn()

    nc.compile()
    return nc


def kernel(time_latent, stems, targets, W_pos, b_pos):
    from concourse.bass_utils import run_bass_kernel_spmd

    # host: positions (tiny linear+sigmoid, fp32 exactly like the reference)
    z = np.einsum("bed,od->beo", time_latent.astype(F32), W_pos.astype(F32))
    z = z.reshape(B, E) + b_pos.reshape(1)[0]
    pos = 1.0 / (1.0 + np.exp(-z, dtype=F32))
    s = (pos * np.float32(N)).astype(np.float64)

    W2cat, e1cat, tinv, e2 = _host_consts()
    n1 = np.arange(N1)
    k2 = np.arange(N2)
    k1 = np.arange(K1)
    T = np.exp(-2j * np.pi * np.outer(n1, k2) / N)   # (n1, k2)
    W1 = np.exp(-2j * np.pi * np.outer(n1, k1) / N1)  # (n1, k1)

    w2cat_b = W2cat.astype(BF16)
    e1cat_b = e1cat.astype(BF16)
    tinv_b = tinv.astype(BF16)
    e2_b = e2.astype(BF16)

    nc = _build_graph()
    in_maps = []
    for c in range(NCORES):
        sl = slice(c * BC, (c + 1) * BC)
        s_flat = s[sl].reshape(-1)                                   # (S,)
        rec = np.empty((S, 128, RECW), dtype=BF16)
        # stems: (S, 256, 128) -> (S, 2, 128, 128) -> (S, 128, 2, 128)
        st = stems[sl].reshape(S, 2, 128, 128).transpose(0, 2, 1, 3)
        rec[:, :, 0:256] = st.reshape(S, 128, 256).astype(BF16)
        A = np.exp(-2j * np.pi * np.outer(s_flat, k2) / N)           # (S, k2)
        C = T[None, :, :] * A[:, None, :]                            # (S, n1, k2)
        rec[:, :, 256:512] = C.real.astype(BF16)
        rec[:, :, 512:768] = C.imag.astype(BF16)
        del C
        Bt = np.exp(-2j * np.pi * np.outer(s_flat, k1) / N1)         # (S, k1)
        M = W1[None, :, :] * Bt[:, None, :]                          # (S, n1, k1)
        Mcat = np.concatenate([M.real, M.imag], 2).astype(BF16)      # (S, 128, 128)
        rec[:, :, 768:896] = Mcat
        rec[:, :, 896:1024] = -Mcat
        del M, Mcat
        in_maps.append({
            "rec": rec.reshape(BC, E, 128, RECW),
            "w2cat": w2cat_b,
            "e1cat": e1cat_b,
            "tinv": tinv_b,
            "e2": e2_b,
        })

    import os
    trace = bool(int(os.environ.get("ATHENA_TRACE", "0")))
    res = run_bass_kernel_spmd(nc, in_maps, core_ids=list(range(NCORES)), trace=trace)
    if trace:
        print(f"HW exec time: {res.exec_time_ns} ns")

    # exact DC / Nyquist corrections on host (the device spectrum covers
    # k1 = 0..63 only): X0 = sum_e sum_n x, XNyq = sum_e cos(pi s) sum_n x(-1)^n
    sign = np.where(np.arange(N) % 2 == 0, 1.0, -1.0).astype(F32)
    sums = stems.astype(F32).sum(axis=2)                       # (B, E)
    dots = (stems.astype(F32) * sign[None, None, :]).sum(axis=2)
    X0 = sums.sum(axis=1)                                      # (B,)
    XNyq = (np.cos(np.pi * s) * dots).sum(axis=1).astype(F32)  # (B,)

    outs = []
    for c in range(NCORES):
        sl = slice(c * BC, (c + 1) * BC)
        y = res.results[c]["out"].reshape(BC, N).astype(F32)
        y = y + (-X0[sl, None].astype(F32)
                 + sign[None, :] * XNyq[sl, None]) / np.float32(N)
        outs.append(y)
    return np.concatenate(outs, 0).reshape(B, 1, N).astype(F32)
